# revision 52
# baseline (speedup 1.0000x reference)
"""Multi-head attention (B=2, S=2048, D=1024, H=16) on 8 trn2 cores.

Sharding: core c handles batch b = c//4 and heads 4g..4g+3 where g = c%4
(tensor-parallel on heads: Wq/Wk/Wv column-sharded, Wpost row-sharded).
Each core emits a partial [S, D] output; host sums the 4 partials per batch
and adds bpost.

v2 pipeline: one continuous exp-paced stream. The ScalarE exp of the
4*2048*2048 score matrix (~147us at 1 elem/lane/cycle) is the hard floor;
everything else (projections, v, AV, post) is slotted into PE slack around
it. Host packs all DRAM params into [128, F] tile-major layouts so input
DMA is ~20 large contiguous transfers issued in deadline order. Both head
pairs are projected once from block-resident x (no re-read). Scores for the
two heads of a pair run as concurrent row-tiled matmuls (K=64 halves of the
PE array). se (exp scores) lives in a 16-deep ring of [128,1024] tiles per
head so exp never waits on AV buffer recycling.
"""

import os

import numpy as np
import ml_dtypes

import concourse.bass as bass
import concourse.tile as tile
from concourse import bacc
from concourse import mybir
from concourse.bass_utils import run_bass_kernel_spmd

F32 = mybir.dt.float32
BF16 = mybir.dt.bfloat16

B, S, D, H = 2, 2048, 1024, 16
DK = D // H          # 64
HPC = 4              # heads per core
DCORE = HPC * DK     # 256 output dims per core
GW = DK + 4          # padded per-head group width in v_aug (64 v + 1 ones + 3 pad)
NKT = D // 128       # 8 contraction tiles over d_in
NMT = S // 128       # 16 token tiles
QB = 512             # query block
NQB = S // QB        # 4
NKV = S // 128       # 16 kv tiles
NJ = NKV // 2        # 8 kv-pair chunks per unit
XBW = NKT * QB       # 4096 packed x columns per 512-token block

_CACHE = {}
LAST_RESULTS = None


def _ensure_ntff_hook():
    """The agent image's antenv lacks axon_hooks; synthesize it and register
    the ctypes NTFF profiling hook so trace=True yields exec times."""
    import sys
    import types

    try:
        from antenv import axon_hooks  # noqa: F401
        return
    except ImportError:
        pass
    mod = types.ModuleType("antenv.axon_hooks")
    _state = {"hook": None}
    mod.set_axon_ntff_profile_hook = lambda h: _state.__setitem__("hook", h)
    mod.get_axon_ntff_profile_hook = lambda: _state["hook"]
    sys.modules["antenv.axon_hooks"] = mod
    import antenv

    antenv.axon_hooks = mod
    try:
        import trn_agent_boot.trn_boot as _tb

        hook = _tb._ntff_profile_via_ctypes("/opt/axon/libaxon_pjrt.so")
        mod.set_axon_ntff_profile_hook(hook)
    except Exception:
        pass


def _build(with_mask: bool):
    nc = bacc.Bacc(None, target_bir_lowering=False)

    # packed DRAM params (host lays everything out tile-major, see _prepare)
    xq_d = nc.declare_dram_parameter("xq", [128, NQB * XBW], BF16, isOutput=False)
    xk_d = nc.declare_dram_parameter("xk", [128, NQB * XBW], BF16, isOutput=False)
    xv_d = nc.declare_dram_parameter("xv", [128, NQB * XBW], BF16, isOutput=False)
    wq_d = nc.declare_dram_parameter("wq", [128, NKT * DCORE], BF16, isOutput=False)
    wk_d = nc.declare_dram_parameter("wk", [128, NKT * DCORE], BF16, isOutput=False)
    wv_d = nc.declare_dram_parameter("wv", [128, NKT * HPC * GW], BF16, isOutput=False)
    ov_d = nc.declare_dram_parameter("ov", [1, HPC * GW], BF16, isOutput=False)
    wp_d = nc.declare_dram_parameter("wp", [128, 2 * D], BF16, isOutput=False)
    bqs = nc.declare_dram_parameter("bqs", [128, 2], F32, isOutput=False)
    bks = nc.declare_dram_parameter("bks", [128, 2], F32, isOutput=False)
    maskT = None
    if with_mask:
        maskT = nc.declare_dram_parameter("maskT", [S, S], F32, isOutput=False)
    out_d = nc.declare_dram_parameter("out_p", [S, D], F32, isOutput=True)

    with tile.TileContext(nc) as tc:
        with (
            tc.tile_pool(name="persist", bufs=1) as persist,
            tc.tile_pool(name="wpool", bufs=1) as wpool,
            tc.tile_pool(name="xkp", bufs=2) as xkp,
            tc.tile_pool(name="xqp", bufs=2) as xqp,
            tc.tile_pool(name="xvp", bufs=3) as xvp,
            tc.tile_pool(name="sexp", bufs=18) as sexp,
            tc.tile_pool(name="small", bufs=2) as small,
            tc.tile_pool(name="outs", bufs=3) as outs,
            tc.tile_pool(name="mpool", bufs=4) as mpool,
            tc.tile_pool(name="pss", bufs=1, space="PSUM") as pss,
            tc.tile_pool(name="pso", bufs=1, space="PSUM") as pso,
            tc.tile_pool(name="mix", bufs=2, space="PSUM") as mix,
        ):
            # ---- ACT table pre-warm (no DMA dependency) ----
            ones_sb = persist.tile([1, 128], BF16, tag="ones", name="ones")
            nc.vector.memset(ones_sb, 1.0)
            warm = small.tile([1, 128], F32, tag="warm", name="warm")
            nc.scalar.activation(
                out=warm, in_=ones_sb, func=mybir.ActivationFunctionType.Exp
            )


            # ---- x block tiles; DMAs emitted in deadline order ----
            xk_t = {}
            xq_t = {}
            xv_t = {}

            def dma_x(store, pool, src, nb, tag, split=1, eng=None):
                t = pool.tile([128, XBW], BF16, tag=tag, name=f"{tag}{nb}")
                hw = XBW // split
                for h in range(split):
                    (eng or nc.sync).dma_start(
                        out=t[:, hw * h : hw * (h + 1)],
                        in_=src[:, XBW * nb + hw * h : XBW * nb + hw * (h + 1)],
                    )
                store[nb] = t

            # Startup uses BOTH HWDGE rings: k-side inputs on the Sync ring,
            # q-side on the Scalar ring, so the two transfer chains overlap.
            # Tiny bias DMAs go first (they'd otherwise queue behind MBs of x
            # and stall the first bias-add); wk/wq are split in halves so the
            # first 4 projection matmuls start after only ~0.75 MB.
            bq_sb = persist.tile([128, 2], F32, tag="bq", name="bq")
            nc.scalar.dma_start(out=bq_sb, in_=bqs[:, :])
            bk_sb = persist.tile([128, 2], F32, tag="bk", name="bk")
            nc.sync.dma_start(out=bk_sb, in_=bks[:, :])
            HW2 = NKT * DCORE // 2
            wk_sb = wpool.tile([128, NKT * DCORE], BF16, tag="wk", name="wk")
            nc.sync.dma_start(out=wk_sb[:, :HW2], in_=wk_d[:, :HW2])
            wq_sb = wpool.tile([128, NKT * DCORE], BF16, tag="wq", name="wq")
            nc.scalar.dma_start(out=wq_sb[:, :HW2], in_=wq_d[:, :HW2])
            xk0 = xkp.tile([128, XBW], BF16, tag="xk", name="xk0")
            nc.sync.dma_start(out=xk0[:, : XBW // 2], in_=xk_d[:, : XBW // 2])
            xk_t[0] = xk0
            xq0 = xqp.tile([128, XBW], BF16, tag="xq", name="xq0")
            nc.scalar.dma_start(out=xq0[:, : XBW // 2], in_=xq_d[:, : XBW // 2])
            xq_t[0] = xq0
            nc.sync.dma_start(out=wk_sb[:, HW2:], in_=wk_d[:, HW2:])
            nc.scalar.dma_start(out=wq_sb[:, HW2:], in_=wq_d[:, HW2:])
            nc.sync.dma_start(out=xk0[:, XBW // 2 :], in_=xk_d[:, XBW // 2 : XBW])
            nc.scalar.dma_start(out=xq0[:, XBW // 2 :], in_=xq_d[:, XBW // 2 : XBW])
            dma_x(xk_t, xkp, xk_d, 1, "xk")
            dma_x(xq_t, xqp, xq_d, 1, "xq", eng=nc.scalar)
            wv_sb = wpool.tile([128, NKT * HPC * GW], BF16, tag="wv", name="wv")
            nc.scalar.dma_start(out=wv_sb, in_=wv_d[:, :])
            ov_sb = persist.tile([1, HPC * GW], BF16, tag="ov", name="ov")
            nc.scalar.dma_start(out=ov_sb, in_=ov_d[:, :])
            dma_x(xv_t, xvp, xv_d, 0, "xv")
            dma_x(xv_t, xvp, xv_d, 1, "xv")
            dma_x(xv_t, xvp, xv_d, 2, "xv")
            # xk2/xk3, xq2/xq3, xv3 (all recycle earlier buffers) and wp are
            # emitted later, after the readers of the buffers they reuse, to
            # keep the HWDGE FIFO from stalling.

            # ---- resident activations ----
            qT_sb = [persist.tile([128, S], BF16, tag=f"qT{p}", name=f"qT{p}") for p in range(2)]
            kT_sb = [persist.tile([128, S], BF16, tag=f"kT{p}", name=f"kT{p}") for p in range(2)]
            v_aug = persist.tile([128, NMT * HPC * GW], BF16, tag="vaug", name="vaug")
            otn_sb = [persist.tile([128, S], BF16, tag=f"otn{p}", name=f"otn{p}") for p in range(2)]

            def proj_block(which, p, nb):
                """qT/kT for head-pair p, 512-token block nb, from packed x."""
                w_sb, x_t, dst, b_sb = (
                    (wq_sb, xq_t, qT_sb, bq_sb)
                    if which == "q"
                    else (wk_sb, xk_t, kT_sb, bk_sb)
                )
                tb = slice(QB * nb, QB * (nb + 1))
                ps = mix.tile([128, QB], F32, tag="mix", name="psproj")
                for kt in range(NKT):
                    nc.tensor.matmul(
                        ps,
                        w_sb[:, kt * DCORE + 128 * p : kt * DCORE + 128 * (p + 1)],
                        x_t[nb][:, QB * kt : QB * (kt + 1)],
                        start=(kt == 0),
                        stop=(kt == NKT - 1),
                    )
                nc.vector.tensor_scalar_add(dst[p][:, tb], ps, b_sb[:, p : p + 1])

            def v_tile(m):
                """one 128-token tile of v_aug (ones column via the ov row;
                bv itself is folded into the host-side bias add)."""
                nb, c0 = m // 4, (m % 4) * 128
                ps_v = mix.tile([128, QB], F32, tag="mix", name="psv")
                nc.tensor.matmul(
                    ps_v[:, : HPC * GW], ones_sb[:, :], ov_sb[:, :],
                    start=True, stop=False,
                )
                for kt in range(NKT):
                    nc.tensor.matmul(
                        ps_v[:, : HPC * GW],
                        xv_t[nb][:, QB * kt + c0 : QB * kt + c0 + 128],
                        wv_sb[:, (HPC * GW) * kt : (HPC * GW) * (kt + 1)],
                        start=False,
                        stop=(kt == NKT - 1),
                    )
                nc.vector.tensor_copy(
                    out=v_aug[:, HPC * GW * m : HPC * GW * (m + 1)],
                    in_=ps_v[:, : HPC * GW],
                )

            # se ring: slot (u*NJ + j) % 16 per head tag
            def se_slot(a, u, j):
                return sexp.tile([128, 1024], BF16, tag=f"se{a}", name=f"se{a}")

            se_ring = {}  # (u, j, a) -> tile

            def scores_step(u, j):
                """scores+exp for unit u=(p,qb), kv pair (2j, 2j+1), both heads."""
                p, qb = divmod(u, NQB)
                qs = slice(QB * qb, QB * (qb + 1))
                regs = [
                    pss.tile([128, 1024], F32, tag=f"R{a}", name=f"R{a}")
                    for a in range(2)
                ]
                for i in range(2):
                    kv = 2 * j + i
                    for a in range(2):
                        hs = slice(64 * a, 64 * (a + 1))
                        nc.tensor.matmul(
                            regs[a][:, 512 * i : 512 * (i + 1)],
                            kT_sb[p][hs, 128 * kv : 128 * (kv + 1)],
                            qT_sb[p][hs, qs],
                            start=True,
                            stop=True,
                            tile_position=(64 * a, 0),
                        )
                if with_mask:
                    for i in range(2):
                        kv = 2 * j + i
                        mt = mpool.tile([128, QB], F32, tag="mask", name="maskt")
                        nc.sync.dma_start(
                            out=mt, in_=maskT[128 * kv : 128 * (kv + 1), qs]
                        )
                        for a in range(2):
                            nc.vector.tensor_add(
                                regs[a][:, 512 * i : 512 * (i + 1)],
                                regs[a][:, 512 * i : 512 * (i + 1)],
                                mt,
                            )
                for a in range(2):
                    t = se_slot(a, u, j)
                    se_ring[(u, j, a)] = t
                    nc.scalar.activation(
                        out=t, in_=regs[a],
                        func=mybir.ActivationFunctionType.Exp,
                    )

            av_ps = {}

            def av_chunk(u, j, on_mix=False):
                """AV accumulation members for kv pair (2j, 2j+1), both heads."""
                p, qb = divmod(u, NQB)
                for a in range(2):
                    hc = 2 * p + a
                    if j == 0:
                        if on_mix:
                            # final unit borrows the (idle) mix bank pair so
                            # it can run concurrently with the previous unit
                            av_ps[(u, a)] = mix.tile(
                                [128, QB], F32, tag="mix", name="psoM"
                            )
                        else:
                            av_ps[(u, a)] = pso.tile(
                                [65, QB], F32, tag=f"pso{a}", name=f"pso{a}"
                            )
                    ps_o = av_ps[(u, a)]
                    se_t = se_ring[(u, j, a)]
                    for i in range(2):
                        kv = 2 * j + i
                        vsl = v_aug[
                            :, GW * (HPC * kv + hc) : GW * (HPC * kv + hc) + 65
                        ]
                        nc.tensor.matmul(
                            ps_o[0:65, :],
                            vsl,
                            se_t[:, 512 * i : 512 * (i + 1)],
                            start=(j == 0 and i == 0),
                            stop=(j == NJ - 1 and i == 1),
                        )

            def av_norm(u):
                """normalize unit u's AV accumulators into otn."""
                p, qb = divmod(u, NQB)
                qs = slice(QB * qb, QB * (qb + 1))
                for a in range(2):
                    ps_o = av_ps.pop((u, a))
                    # one copy releases the PSUM accumulator immediately so
                    # the next unit's AV chain isn't stalled behind the norm
                    avs = small.tile([65, QB], F32, tag="avs", name="avs")
                    nc.vector.tensor_copy(out=avs, in_=ps_o[0:65, :])
                    zrow = small.tile([1, QB], F32, tag="zrow", name="zrow")
                    nc.vector.tensor_copy(out=zrow, in_=avs[64:65, :])
                    rc = small.tile([1, QB], F32, tag="rc", name="rc")
                    nc.vector.reciprocal_approx_fast(out=rc, in_=zrow[:, :])
                    bc = small.tile([64, QB], F32, tag="bc", name="bc")
                    nc.gpsimd.partition_broadcast(bc, rc[:, :])
                    nc.vector.tensor_mul(
                        otn_sb[p][64 * a : 64 * (a + 1), qs],
                        avs[0:64, :],
                        bc,
                    )

            def post_block(qb):
                """post projection + output DMA for one q-block."""
                for mi in range(QB // 128):
                    m = (QB * qb) // 128 + mi
                    ms = slice(128 * m, 128 * (m + 1))
                    o_t = outs.tile([128, D], F32, tag="outp", name="outp")
                    for nj in range(2):
                        ps_p = mix.tile([128, 512], F32, tag="mix", name="psp")
                        for kp in range(2):
                            nc.tensor.matmul(
                                ps_p,
                                otn_sb[kp][:, ms],
                                wp_box["wp"][:, D * kp + 512 * nj : D * kp + 512 * (nj + 1)],
                                start=(kp == 0),
                                stop=(kp == 1),
                            )
                        nc.vector.tensor_copy(
                            out=o_t[:, 512 * nj : 512 * (nj + 1)], in_=ps_p
                        )
                    nc.sync.dma_start(out=out_d[ms, :], in_=o_t)

            # ================= emission schedule =================
            # lead-in: get the exp stream started on unit 0 ASAP.
            # k-projections for BOTH pairs happen per block so the xk pool
            # (bufs=2) can recycle: xk2/xk3 DMAs are emitted only after all
            # readers of the buffer they reuse.
            proj_block("k", 0, 0)
            proj_block("q", 0, 0)
            scores_step(0, 0)
            scores_step(0, 1)
            proj_block("k", 0, 1)
            proj_block("k", 1, 0)
            scores_step(0, 2)
            scores_step(0, 3)
            proj_block("k", 1, 1)
            dma_x(xk_t, xkp, xk_d, 2, "xk")
            proj_block("k", 0, 2)
            scores_step(0, 4)
            scores_step(0, 5)
            proj_block("k", 1, 2)
            dma_x(xk_t, xkp, xk_d, 3, "xk")
            proj_block("k", 0, 3)
            scores_step(0, 6)
            scores_step(0, 7)
            proj_block("k", 1, 3)
            proj_block("q", 0, 1)

            # period 0: scores U1; fillers: remaining projections + v b0/b1.
            # (CALL, emit-a-DMA) pairs; DMAs sit at the right FIFO position.
            def fillers_p0():
                yield lambda: proj_block("q", 1, 0)
                # xq2 recycles xq0's buffer (readers: q p0 b0, q p1 b0)
                yield lambda: dma_x(xq_t, xqp, xq_d, 2, "xq")
                for m in range(0, 4):
                    yield (lambda m=m: v_tile(m))
                # xv3 recycles xv0's buffer (readers: v tiles 0-3)
                yield lambda: dma_x(xv_t, xvp, xv_d, 3, "xv")
                yield lambda: self_wp()
                yield lambda: proj_block("q", 1, 1)
                # xq3 recycles xq1's buffer (readers: q p0 b1, q p1 b1)
                yield lambda: dma_x(xq_t, xqp, xq_d, 3, "xq")
                for m in range(4, 8):
                    yield (lambda m=m: v_tile(m))
                yield lambda: proj_block("q", 0, 2)
                yield lambda: proj_block("q", 0, 3)

            wp_box = {}

            def self_wp():
                t = wpool.tile([128, 2 * D], BF16, tag="wp", name="wp")
                nc.sync.dma_start(out=t, in_=wp_d[:, :])
                wp_box["wp"] = t

            fl = list(fillers_p0())
            fi = 0
            for j in range(NJ):
                scores_step(1, j)
                take = (len(fl) * (j + 1)) // NJ
                while fi < take:
                    fl[fi]()
                    fi += 1

            # periods 1..7: lag-0 AV — unit t's AV runs in the same period as
            # its exps (which pace it), so norms/posts complete in-stream and
            # only norm U7 + post(3) remain after the last exp.
            # U0's AV waits for v (lands mid-period-1) so it runs lag-1 in
            # period 1; U1's AV catches up as a burst at the start of period
            # 2 (its exps finished in period 1).
            # Scores go first in each j-step; the 18-deep se ring guarantees
            # the slot a scores step recycles was read by an AV chunk at
            # least 2 j-steps earlier in emission order.
            extras = {
                1: [(lambda m=m: v_tile(m)) for m in range(8, 16)],
                3: [lambda: proj_block("q", 1, 2)],
                4: [lambda: proj_block("q", 1, 3)],
            }
            for t in range(1, 8):
                us = t + 1  # scores unit
                if t == 2:
                    for j in range(NJ):
                        av_chunk(1, j)
                    av_norm(1)
                ua = 0 if t == 1 else t  # AV unit this period
                ext = extras.get(t, [])
                ei = 0
                # pre-loop: anything AV chunk 0 needs (v tiles 2j, 2j+1)
                take = (len(ext) * 2) // NJ
                while ei < take:
                    ext[ei]()
                    ei += 1
                for j in range(NJ):
                    take = min(len(ext), (len(ext) * (j + 3)) // NJ)
                    while ei < take:
                        ext[ei]()
                        ei += 1
                    if us < 8:
                        scores_step(us, j)
                    av_chunk(ua, j)
                av_norm(ua)
                if ua >= 4:
                    post_block(ua - 4)

    nc.compile()
    return nc


def _get_program(with_mask: bool):
    if with_mask not in _CACHE:
        _CACHE[with_mask] = _build(with_mask)
    return _CACHE[with_mask]


def _pack_rows(arr, bf16):
    """[8*128, F] -> [128, 8*F] tile-major (kt-major in free dim)."""
    kt, f = arr.shape[0] // 128, arr.shape[1]
    return np.ascontiguousarray(
        arr.reshape(kt, 128, f).transpose(1, 0, 2).reshape(128, kt * f)
    ).astype(bf16)


def _pack_x(x, bf16):
    """x [S, D] -> packed [128, NQB*XBW]: block nb, then kt, then token."""
    xT = x.T.astype(np.float32)  # [D, S]
    a = xT.reshape(NKT, 128, NQB, QB).transpose(1, 2, 0, 3)  # [128, nb, kt, c]
    return np.ascontiguousarray(a.reshape(128, NQB * XBW)).astype(bf16)


def _prepare(query, key, value, mask, Wq, bq, Wk, bk, Wv, bv, Wpost, bpost,
             per_dim_scale):
    f32 = np.float32
    query = np.asarray(query, f32)
    key = np.asarray(key, f32)
    value = np.asarray(value, f32)
    mask = np.asarray(mask, f32)
    Wq = np.asarray(Wq, f32)
    bq = np.asarray(bq, f32)
    Wk = np.asarray(Wk, f32)
    bk = np.asarray(bk, f32)
    Wv = np.asarray(Wv, f32)
    bv = np.asarray(bv, f32)
    Wpost = np.asarray(Wpost, f32)
    bpost = np.asarray(bpost, f32)
    per_dim_scale = np.asarray(per_dim_scale, f32)

    r_softplus_0 = 1.442695041
    scale = (r_softplus_0 / np.sqrt(DK)) * np.log1p(np.exp(per_dim_scale))
    scale = scale.astype(f32)  # [DK]
    scale_tiled = np.tile(scale, HPC)  # [DCORE]

    with_mask = bool(np.any(mask))
    nc = _get_program(with_mask)

    bf16 = ml_dtypes.bfloat16
    in_maps = []
    for c in range(8):
        b = c // 4
        g = c % 4
        dsl = slice(DCORE * g, DCORE * (g + 1))

        wqT_s = Wq[dsl, :].T * scale_tiled[None, :]  # [D, 256] f32
        wkT_s = Wk[dsl, :].T
        wvT_s = Wv[dsl, :].T  # [D, 256]
        wvT_pad = np.zeros((D, HPC * GW), f32)
        ov = np.zeros((1, HPC * GW), f32)
        for hc in range(HPC):
            wvT_pad[:, GW * hc : GW * hc + DK] = wvT_s[:, DK * hc : DK * (hc + 1)]
            ov[0, GW * hc + DK] = 1.0
        wpT_s = Wpost[:, dsl].T  # [256, 1024]

        m = {
            "xq": _pack_x(query[b], bf16),
            "xk": _pack_x(key[b], bf16),
            "xv": _pack_x(value[b], bf16),
            "wq": _pack_rows(wqT_s, bf16),
            "wk": _pack_rows(wkT_s, bf16),
            "wv": _pack_rows(wvT_pad, bf16),
            "wp": _pack_rows(wpT_s, bf16),
            "ov": ov.astype(bf16),
            "bqs": np.ascontiguousarray(
                (bq[dsl] * scale_tiled).reshape(2, 128).T
            ).astype(f32),
            "bks": np.ascontiguousarray(bk[dsl].reshape(2, 128).T).astype(f32),
        }
        if with_mask:
            m["maskT"] = np.ascontiguousarray(mask[0, 0].T)
        in_maps.append(m)

    return nc, in_maps, bpost


def kernel(query, key, value, mask, Wq, bq, Wk, bk, Wv, bv, Wpost, bpost,
           per_dim_scale):
    global LAST_RESULTS
    nc, in_maps, bpost = _prepare(
        query, key, value, mask, Wq, bq, Wk, bk, Wv, bv, Wpost, bpost,
        per_dim_scale,
    )
    trace = os.environ.get("BASS_TRACE", "") not in ("", "0")
    if trace:
        _ensure_ntff_hook()
    res = run_bass_kernel_spmd(nc, in_maps, list(range(8)), trace=trace)
    LAST_RESULTS = res

    out = np.zeros((B, S, D), np.float32)
    for c in range(8):
        out[c // 4] += np.asarray(res.results[c]["out_p"], np.float32)
    # softmax rows sum to 1, so the value-projection bias contributes the
    # constant vector bv @ Wpost^T to every output row (folded here).
    bias = np.asarray(bpost, np.float32) + np.asarray(bv, np.float32) @ np.asarray(
        Wpost, np.float32
    ).T
    out += bias[None, None, :]
    return out


# revision 55
# speedup vs baseline: 1.0168x; 1.0168x over previous
"""Multi-head attention (B=2, S=2048, D=1024, H=16) on 8 trn2 cores.

Sharding: core c handles batch b = c//4 and heads 4g..4g+3 where g = c%4
(tensor-parallel on heads: Wq/Wk/Wv column-sharded, Wpost row-sharded).
Each core emits a partial [S, D] output; host sums the 4 partials per batch
and adds bpost.

v2 pipeline: one continuous exp-paced stream. The ScalarE exp of the
4*2048*2048 score matrix (~147us at 1 elem/lane/cycle) is the hard floor;
everything else (projections, v, AV, post) is slotted into PE slack around
it. Host packs all DRAM params into [128, F] tile-major layouts so input
DMA is ~20 large contiguous transfers issued in deadline order. Both head
pairs are projected once from block-resident x (no re-read). Scores for the
two heads of a pair run as concurrent row-tiled matmuls (K=64 halves of the
PE array). se (exp scores) lives in a 16-deep ring of [128,1024] tiles per
head so exp never waits on AV buffer recycling.
"""

import os

import numpy as np
import ml_dtypes

import concourse.bass as bass
import concourse.tile as tile
from concourse import bacc
from concourse import mybir
from concourse.bass_utils import run_bass_kernel_spmd

F32 = mybir.dt.float32
BF16 = mybir.dt.bfloat16

B, S, D, H = 2, 2048, 1024, 16
DK = D // H          # 64
HPC = 4              # heads per core
DCORE = HPC * DK     # 256 output dims per core
GW = DK + 4          # padded per-head group width in v_aug (64 v + 1 ones + 3 pad)
NKT = D // 128       # 8 contraction tiles over d_in
NMT = S // 128       # 16 token tiles
QB = 512             # query block
NQB = S // QB        # 4
NKV = S // 128       # 16 kv tiles
NJ = NKV // 2        # 8 kv-pair chunks per unit
XBW = NKT * QB       # 4096 packed x columns per 512-token block

_CACHE = {}
LAST_RESULTS = None


def _ensure_ntff_hook():
    """The agent image's antenv lacks axon_hooks; synthesize it and register
    the ctypes NTFF profiling hook so trace=True yields exec times."""
    import sys
    import types

    try:
        from antenv import axon_hooks  # noqa: F401
        return
    except ImportError:
        pass
    mod = types.ModuleType("antenv.axon_hooks")
    _state = {"hook": None}
    mod.set_axon_ntff_profile_hook = lambda h: _state.__setitem__("hook", h)
    mod.get_axon_ntff_profile_hook = lambda: _state["hook"]
    sys.modules["antenv.axon_hooks"] = mod
    import antenv

    antenv.axon_hooks = mod
    try:
        import trn_agent_boot.trn_boot as _tb

        hook = _tb._ntff_profile_via_ctypes("/opt/axon/libaxon_pjrt.so")
        mod.set_axon_ntff_profile_hook(hook)
    except Exception:
        pass


def _build(with_mask: bool):
    nc = bacc.Bacc(None, target_bir_lowering=False)

    # packed DRAM params (host lays everything out tile-major, see _prepare)
    xq_d = nc.declare_dram_parameter("xq", [128, NQB * XBW], BF16, isOutput=False)
    xk_d = nc.declare_dram_parameter("xk", [128, NQB * XBW], BF16, isOutput=False)
    xv_d = nc.declare_dram_parameter("xv", [128, NQB * XBW], BF16, isOutput=False)
    wq_d = nc.declare_dram_parameter("wq", [128, NKT * DCORE], BF16, isOutput=False)
    wk_d = nc.declare_dram_parameter("wk", [128, NKT * DCORE], BF16, isOutput=False)
    wv_d = nc.declare_dram_parameter("wv", [128, NKT * HPC * GW], BF16, isOutput=False)
    ov_d = nc.declare_dram_parameter("ov", [1, HPC * GW], BF16, isOutput=False)
    wp_d = nc.declare_dram_parameter("wp", [128, 2 * D], BF16, isOutput=False)
    bqs = nc.declare_dram_parameter("bqs", [128, 2], F32, isOutput=False)
    bks = nc.declare_dram_parameter("bks", [128, 2], F32, isOutput=False)
    maskT = None
    if with_mask:
        maskT = nc.declare_dram_parameter("maskT", [S, S], F32, isOutput=False)
    out_d = nc.declare_dram_parameter("out_p", [S, D], F32, isOutput=True)

    with tile.TileContext(nc) as tc:
        with (
            tc.tile_pool(name="persist", bufs=1) as persist,
            tc.tile_pool(name="wpool", bufs=1) as wpool,
            tc.tile_pool(name="xkp", bufs=2) as xkp,
            tc.tile_pool(name="xqp", bufs=2) as xqp,
            tc.tile_pool(name="xvp", bufs=3) as xvp,
            tc.tile_pool(name="sexp", bufs=(16 if with_mask else 18)) as sexp,
            tc.tile_pool(name="small", bufs=2) as small,
            tc.tile_pool(name="outs", bufs=3) as outs,
            tc.tile_pool(name="mpool", bufs=2) as mpool,
            tc.tile_pool(name="pss", bufs=1, space="PSUM") as pss,
            tc.tile_pool(name="pso", bufs=1, space="PSUM") as pso,
            tc.tile_pool(name="mix", bufs=2, space="PSUM") as mix,
        ):
            # ---- ACT table pre-warm (no DMA dependency) ----
            ones_sb = persist.tile([1, 128], BF16, tag="ones", name="ones")
            nc.vector.memset(ones_sb, 1.0)
            warm = small.tile([1, 128], F32, tag="warm", name="warm")
            nc.scalar.activation(
                out=warm, in_=ones_sb, func=mybir.ActivationFunctionType.Exp
            )


            # ---- x block tiles; DMAs emitted in deadline order ----
            xk_t = {}
            xq_t = {}
            xv_t = {}

            def dma_x(store, pool, src, nb, tag, split=1, eng=None):
                t = pool.tile([128, XBW], BF16, tag=tag, name=f"{tag}{nb}")
                hw = XBW // split
                for h in range(split):
                    (eng or nc.sync).dma_start(
                        out=t[:, hw * h : hw * (h + 1)],
                        in_=src[:, XBW * nb + hw * h : XBW * nb + hw * (h + 1)],
                    )
                store[nb] = t

            # Startup uses BOTH HWDGE rings: k-side inputs on the Sync ring,
            # q-side on the Scalar ring, so the two transfer chains overlap.
            # Tiny bias DMAs go first (they'd otherwise queue behind MBs of x
            # and stall the first bias-add); wk/wq are split in halves so the
            # first 4 projection matmuls start after only ~0.75 MB.
            bq_sb = persist.tile([128, 2], F32, tag="bq", name="bq")
            nc.scalar.dma_start(out=bq_sb, in_=bqs[:, :])
            bk_sb = persist.tile([128, 2], F32, tag="bk", name="bk")
            nc.sync.dma_start(out=bk_sb, in_=bks[:, :])
            HW2 = NKT * DCORE // 2
            wk_sb = wpool.tile([128, NKT * DCORE], BF16, tag="wk", name="wk")
            nc.sync.dma_start(out=wk_sb[:, :HW2], in_=wk_d[:, :HW2])
            wq_sb = wpool.tile([128, NKT * DCORE], BF16, tag="wq", name="wq")
            nc.scalar.dma_start(out=wq_sb[:, :HW2], in_=wq_d[:, :HW2])
            xk0 = xkp.tile([128, XBW], BF16, tag="xk", name="xk0")
            nc.sync.dma_start(out=xk0[:, : XBW // 2], in_=xk_d[:, : XBW // 2])
            xk_t[0] = xk0
            xq0 = xqp.tile([128, XBW], BF16, tag="xq", name="xq0")
            nc.scalar.dma_start(out=xq0[:, : XBW // 2], in_=xq_d[:, : XBW // 2])
            xq_t[0] = xq0
            nc.sync.dma_start(out=wk_sb[:, HW2:], in_=wk_d[:, HW2:])
            nc.scalar.dma_start(out=wq_sb[:, HW2:], in_=wq_d[:, HW2:])
            nc.sync.dma_start(out=xk0[:, XBW // 2 :], in_=xk_d[:, XBW // 2 : XBW])
            nc.scalar.dma_start(out=xq0[:, XBW // 2 :], in_=xq_d[:, XBW // 2 : XBW])
            dma_x(xk_t, xkp, xk_d, 1, "xk")
            dma_x(xq_t, xqp, xq_d, 1, "xq", eng=nc.scalar)
            wv_sb = wpool.tile([128, NKT * HPC * GW], BF16, tag="wv", name="wv")
            nc.scalar.dma_start(out=wv_sb, in_=wv_d[:, :])
            ov_sb = persist.tile([1, HPC * GW], BF16, tag="ov", name="ov")
            nc.scalar.dma_start(out=ov_sb, in_=ov_d[:, :])
            dma_x(xv_t, xvp, xv_d, 0, "xv")
            dma_x(xv_t, xvp, xv_d, 1, "xv")
            dma_x(xv_t, xvp, xv_d, 2, "xv")
            # xk2/xk3, xq2/xq3, xv3 (all recycle earlier buffers) and wp are
            # emitted later, after the readers of the buffers they reuse, to
            # keep the HWDGE FIFO from stalling.

            # ---- resident activations ----
            qT_sb = [persist.tile([128, S], BF16, tag=f"qT{p}", name=f"qT{p}") for p in range(2)]
            kT_sb = [persist.tile([128, S], BF16, tag=f"kT{p}", name=f"kT{p}") for p in range(2)]
            v_aug = persist.tile([128, NMT * HPC * GW], BF16, tag="vaug", name="vaug")
            otn_sb = [persist.tile([128, S], BF16, tag=f"otn{p}", name=f"otn{p}") for p in range(2)]

            def proj_block(which, p, nb):
                """qT/kT for head-pair p, 512-token block nb, from packed x."""
                w_sb, x_t, dst, b_sb = (
                    (wq_sb, xq_t, qT_sb, bq_sb)
                    if which == "q"
                    else (wk_sb, xk_t, kT_sb, bk_sb)
                )
                tb = slice(QB * nb, QB * (nb + 1))
                ps = mix.tile([128, QB], F32, tag="mix", name="psproj")
                for kt in range(NKT):
                    nc.tensor.matmul(
                        ps,
                        w_sb[:, kt * DCORE + 128 * p : kt * DCORE + 128 * (p + 1)],
                        x_t[nb][:, QB * kt : QB * (kt + 1)],
                        start=(kt == 0),
                        stop=(kt == NKT - 1),
                    )
                nc.vector.tensor_scalar_add(dst[p][:, tb], ps, b_sb[:, p : p + 1])

            def v_tile(m):
                """one 128-token tile of v_aug (ones column via the ov row;
                bv itself is folded into the host-side bias add)."""
                nb, c0 = m // 4, (m % 4) * 128
                ps_v = mix.tile([128, QB], F32, tag="mix", name="psv")
                nc.tensor.matmul(
                    ps_v[:, : HPC * GW], ones_sb[:, :], ov_sb[:, :],
                    start=True, stop=False,
                )
                for kt in range(NKT):
                    nc.tensor.matmul(
                        ps_v[:, : HPC * GW],
                        xv_t[nb][:, QB * kt + c0 : QB * kt + c0 + 128],
                        wv_sb[:, (HPC * GW) * kt : (HPC * GW) * (kt + 1)],
                        start=False,
                        stop=(kt == NKT - 1),
                    )
                nc.vector.tensor_copy(
                    out=v_aug[:, HPC * GW * m : HPC * GW * (m + 1)],
                    in_=ps_v[:, : HPC * GW],
                )

            # se ring: slot (u*NJ + j) % 16 per head tag
            def se_slot(a, u, j):
                return sexp.tile([128, 1024], BF16, tag=f"se{a}", name=f"se{a}")

            se_ring = {}  # (u, j, a) -> tile

            def scores_step(u, j):
                """scores+exp for unit u=(p,qb), kv pair (2j, 2j+1), both heads."""
                p, qb = divmod(u, NQB)
                qs = slice(QB * qb, QB * (qb + 1))
                regs = [
                    pss.tile([128, 1024], F32, tag=f"R{a}", name=f"R{a}")
                    for a in range(2)
                ]
                for i in range(2):
                    kv = 2 * j + i
                    for a in range(2):
                        hs = slice(64 * a, 64 * (a + 1))
                        nc.tensor.matmul(
                            regs[a][:, 512 * i : 512 * (i + 1)],
                            kT_sb[p][hs, 128 * kv : 128 * (kv + 1)],
                            qT_sb[p][hs, qs],
                            start=True,
                            stop=True,
                            tile_position=(64 * a, 0),
                        )
                if with_mask:
                    for i in range(2):
                        kv = 2 * j + i
                        mt = mpool.tile([128, QB], F32, tag="mask", name="maskt")
                        nc.sync.dma_start(
                            out=mt, in_=maskT[128 * kv : 128 * (kv + 1), qs]
                        )
                        for a in range(2):
                            nc.vector.tensor_add(
                                regs[a][:, 512 * i : 512 * (i + 1)],
                                regs[a][:, 512 * i : 512 * (i + 1)],
                                mt,
                            )
                for a in range(2):
                    t = se_slot(a, u, j)
                    se_ring[(u, j, a)] = t
                    nc.scalar.activation(
                        out=t, in_=regs[a],
                        func=mybir.ActivationFunctionType.Exp,
                    )

            av_ps = {}

            def av_chunk(u, j, on_mix=False):
                """AV accumulation members for kv pair (2j, 2j+1), both heads."""
                p, qb = divmod(u, NQB)
                for a in range(2):
                    hc = 2 * p + a
                    if j == 0:
                        if on_mix:
                            # final unit borrows the (idle) mix bank pair so
                            # it can run concurrently with the previous unit
                            av_ps[(u, a)] = mix.tile(
                                [128, QB], F32, tag="mix", name="psoM"
                            )
                        else:
                            av_ps[(u, a)] = pso.tile(
                                [65, QB], F32, tag=f"pso{a}", name=f"pso{a}"
                            )
                    ps_o = av_ps[(u, a)]
                    se_t = se_ring[(u, j, a)]
                    for i in range(2):
                        kv = 2 * j + i
                        vsl = v_aug[
                            :, GW * (HPC * kv + hc) : GW * (HPC * kv + hc) + 65
                        ]
                        nc.tensor.matmul(
                            ps_o[0:65, :],
                            vsl,
                            se_t[:, 512 * i : 512 * (i + 1)],
                            start=(j == 0 and i == 0),
                            stop=(j == NJ - 1 and i == 1),
                        )

            def av_norm(u):
                """normalize unit u's AV accumulators into otn."""
                p, qb = divmod(u, NQB)
                qs = slice(QB * qb, QB * (qb + 1))
                for a in range(2):
                    ps_o = av_ps.pop((u, a))
                    # one copy releases the PSUM accumulator immediately so
                    # the next unit's AV chain isn't stalled behind the norm
                    avs = small.tile([65, QB], F32, tag="avs", name="avs")
                    nc.vector.tensor_copy(out=avs, in_=ps_o[0:65, :])
                    zrow = small.tile([1, QB], F32, tag="zrow", name="zrow")
                    nc.vector.tensor_copy(out=zrow, in_=avs[64:65, :])
                    rc = small.tile([1, QB], F32, tag="rc", name="rc")
                    nc.vector.reciprocal_approx_fast(out=rc, in_=zrow[:, :])
                    bc = small.tile([64, QB], F32, tag="bc", name="bc")
                    nc.gpsimd.partition_broadcast(bc, rc[:, :])
                    nc.vector.tensor_mul(
                        otn_sb[p][64 * a : 64 * (a + 1), qs],
                        avs[0:64, :],
                        bc,
                    )

            def post_block(qb):
                """post projection + output DMA for one q-block."""
                for mi in range(QB // 128):
                    m = (QB * qb) // 128 + mi
                    ms = slice(128 * m, 128 * (m + 1))
                    o_t = outs.tile([128, D], F32, tag="outp", name="outp")
                    for nj in range(2):
                        ps_p = mix.tile([128, 512], F32, tag="mix", name="psp")
                        for kp in range(2):
                            nc.tensor.matmul(
                                ps_p,
                                otn_sb[kp][:, ms],
                                wp_box["wp"][:, D * kp + 512 * nj : D * kp + 512 * (nj + 1)],
                                start=(kp == 0),
                                stop=(kp == 1),
                            )
                        nc.vector.tensor_copy(
                            out=o_t[:, 512 * nj : 512 * (nj + 1)], in_=ps_p
                        )
                    nc.sync.dma_start(out=out_d[ms, :], in_=o_t)

            # ================= emission schedule =================
            # lead-in: get the exp stream started on unit 0 ASAP.
            # k-projections for BOTH pairs happen per block so the xk pool
            # (bufs=2) can recycle: xk2/xk3 DMAs are emitted only after all
            # readers of the buffer they reuse.
            proj_block("k", 0, 0)
            proj_block("q", 0, 0)
            scores_step(0, 0)
            scores_step(0, 1)
            proj_block("k", 0, 1)
            proj_block("k", 1, 0)
            scores_step(0, 2)
            scores_step(0, 3)
            proj_block("k", 1, 1)
            dma_x(xk_t, xkp, xk_d, 2, "xk")
            proj_block("k", 0, 2)
            scores_step(0, 4)
            scores_step(0, 5)
            proj_block("k", 1, 2)
            dma_x(xk_t, xkp, xk_d, 3, "xk")
            proj_block("k", 0, 3)
            scores_step(0, 6)
            scores_step(0, 7)
            proj_block("k", 1, 3)
            proj_block("q", 0, 1)

            # period 0: scores U1; fillers: remaining projections + v b0/b1.
            # (CALL, emit-a-DMA) pairs; DMAs sit at the right FIFO position.
            def fillers_p0():
                yield lambda: proj_block("q", 1, 0)
                # xq2 recycles xq0's buffer (readers: q p0 b0, q p1 b0)
                yield lambda: dma_x(xq_t, xqp, xq_d, 2, "xq")
                for m in range(0, 4):
                    yield (lambda m=m: v_tile(m))
                # xv3 recycles xv0's buffer (readers: v tiles 0-3)
                yield lambda: dma_x(xv_t, xvp, xv_d, 3, "xv")
                yield lambda: self_wp()
                yield lambda: proj_block("q", 1, 1)
                # xq3 recycles xq1's buffer (readers: q p0 b1, q p1 b1)
                yield lambda: dma_x(xq_t, xqp, xq_d, 3, "xq")
                for m in range(4, 8):
                    yield (lambda m=m: v_tile(m))
                yield lambda: proj_block("q", 0, 2)
                yield lambda: proj_block("q", 0, 3)

            wp_box = {}

            def self_wp():
                t = wpool.tile([128, 2 * D], BF16, tag="wp", name="wp")
                nc.sync.dma_start(out=t, in_=wp_d[:, :])
                wp_box["wp"] = t

            fl = list(fillers_p0())
            fi = 0
            for j in range(NJ):
                scores_step(1, j)
                take = (len(fl) * (j + 1)) // NJ
                while fi < take:
                    fl[fi]()
                    fi += 1

            # periods 1..7: lag-0 AV — unit t's AV runs in the same period as
            # its exps (which pace it), so norms/posts complete in-stream and
            # only norm U7 + post(3) remain after the last exp.
            # U0's AV waits for v (lands mid-period-1) so it runs lag-1 in
            # period 1; U1's AV catches up as a burst at the start of period
            # 2 (its exps finished in period 1).
            # Scores go first in each j-step; the 18-deep se ring guarantees
            # the slot a scores step recycles was read by an AV chunk at
            # least 2 j-steps earlier in emission order.
            extras = {
                1: [(lambda m=m: v_tile(m)) for m in range(8, 16)],
                3: [lambda: proj_block("q", 1, 2)],
                4: [lambda: proj_block("q", 1, 3)],
            }
            for t in range(1, 8):
                us = t + 1  # scores unit
                if t == 2:
                    for j in range(NJ):
                        av_chunk(1, j)
                    av_norm(1)
                ua = 0 if t == 1 else t  # AV unit this period
                ext = extras.get(t, [])
                ei = 0
                # pre-loop: anything AV chunk 0 needs (v tiles 2j, 2j+1)
                take = (len(ext) * 2) // NJ
                while ei < take:
                    ext[ei]()
                    ei += 1
                for j in range(NJ):
                    take = min(len(ext), (len(ext) * (j + 3)) // NJ)
                    while ei < take:
                        ext[ei]()
                        ei += 1
                    # 16-deep se ring (mask build): the slot a scores step
                    # recycles is read by the same-j AV chunk, so AV must be
                    # emitted first. 18-deep ring: scores first is safe and
                    # lets the matmuls fire as soon as a region frees.
                    if with_mask:
                        av_chunk(ua, j)
                        if us < 8:
                            scores_step(us, j)
                    else:
                        if us < 8:
                            scores_step(us, j)
                        av_chunk(ua, j)
                av_norm(ua)
                if ua >= 4:
                    post_block(ua - 4)

    nc.compile()
    return nc


def _get_program(with_mask: bool):
    if with_mask not in _CACHE:
        _CACHE[with_mask] = _build(with_mask)
    return _CACHE[with_mask]


def _pack_rows(arr, bf16):
    """[8*128, F] -> [128, 8*F] tile-major (kt-major in free dim)."""
    kt, f = arr.shape[0] // 128, arr.shape[1]
    return np.ascontiguousarray(
        arr.reshape(kt, 128, f).transpose(1, 0, 2).reshape(128, kt * f)
    ).astype(bf16)


def _pack_x(x, bf16):
    """x [S, D] -> packed [128, NQB*XBW]: block nb, then kt, then token."""
    xT = x.T.astype(np.float32)  # [D, S]
    a = xT.reshape(NKT, 128, NQB, QB).transpose(1, 2, 0, 3)  # [128, nb, kt, c]
    return np.ascontiguousarray(a.reshape(128, NQB * XBW)).astype(bf16)


def _prepare(query, key, value, mask, Wq, bq, Wk, bk, Wv, bv, Wpost, bpost,
             per_dim_scale):
    f32 = np.float32
    query = np.asarray(query, f32)
    key = np.asarray(key, f32)
    value = np.asarray(value, f32)
    mask = np.asarray(mask, f32)
    Wq = np.asarray(Wq, f32)
    bq = np.asarray(bq, f32)
    Wk = np.asarray(Wk, f32)
    bk = np.asarray(bk, f32)
    Wv = np.asarray(Wv, f32)
    bv = np.asarray(bv, f32)
    Wpost = np.asarray(Wpost, f32)
    bpost = np.asarray(bpost, f32)
    per_dim_scale = np.asarray(per_dim_scale, f32)

    r_softplus_0 = 1.442695041
    scale = (r_softplus_0 / np.sqrt(DK)) * np.log1p(np.exp(per_dim_scale))
    scale = scale.astype(f32)  # [DK]
    scale_tiled = np.tile(scale, HPC)  # [DCORE]

    with_mask = bool(np.any(mask))
    nc = _get_program(with_mask)

    bf16 = ml_dtypes.bfloat16
    in_maps = []
    for c in range(8):
        b = c // 4
        g = c % 4
        dsl = slice(DCORE * g, DCORE * (g + 1))

        wqT_s = Wq[dsl, :].T * scale_tiled[None, :]  # [D, 256] f32
        wkT_s = Wk[dsl, :].T
        wvT_s = Wv[dsl, :].T  # [D, 256]
        wvT_pad = np.zeros((D, HPC * GW), f32)
        ov = np.zeros((1, HPC * GW), f32)
        for hc in range(HPC):
            wvT_pad[:, GW * hc : GW * hc + DK] = wvT_s[:, DK * hc : DK * (hc + 1)]
            ov[0, GW * hc + DK] = 1.0
        wpT_s = Wpost[:, dsl].T  # [256, 1024]

        m = {
            "xq": _pack_x(query[b], bf16),
            "xk": _pack_x(key[b], bf16),
            "xv": _pack_x(value[b], bf16),
            "wq": _pack_rows(wqT_s, bf16),
            "wk": _pack_rows(wkT_s, bf16),
            "wv": _pack_rows(wvT_pad, bf16),
            "wp": _pack_rows(wpT_s, bf16),
            "ov": ov.astype(bf16),
            "bqs": np.ascontiguousarray(
                (bq[dsl] * scale_tiled).reshape(2, 128).T
            ).astype(f32),
            "bks": np.ascontiguousarray(bk[dsl].reshape(2, 128).T).astype(f32),
        }
        if with_mask:
            m["maskT"] = np.ascontiguousarray(mask[0, 0].T)
        in_maps.append(m)

    return nc, in_maps, bpost


def kernel(query, key, value, mask, Wq, bq, Wk, bk, Wv, bv, Wpost, bpost,
           per_dim_scale):
    global LAST_RESULTS
    nc, in_maps, bpost = _prepare(
        query, key, value, mask, Wq, bq, Wk, bk, Wv, bv, Wpost, bpost,
        per_dim_scale,
    )
    trace = os.environ.get("BASS_TRACE", "") not in ("", "0")
    if trace:
        _ensure_ntff_hook()
    res = run_bass_kernel_spmd(nc, in_maps, list(range(8)), trace=trace)
    LAST_RESULTS = res

    out = np.zeros((B, S, D), np.float32)
    for c in range(8):
        out[c // 4] += np.asarray(res.results[c]["out_p"], np.float32)
    # softmax rows sum to 1, so the value-projection bias contributes the
    # constant vector bv @ Wpost^T to every output row (folded here).
    bias = np.asarray(bpost, np.float32) + np.asarray(bv, np.float32) @ np.asarray(
        Wpost, np.float32
    ).T
    out += bias[None, None, :]
    return out
